# revision 1
# baseline (speedup 1.0000x reference)
"""DTW frames layer on 8 Trainium2 NeuronCores.

Reference computation (per (n, k) problem):
    cost[p, w] = max(0, ||x[n, :, w] - patts[k, :, p]||^2)          (P=32, W=128)
    dtw[0, w]  = cumsum_w cost[0, w]
    dtw[p, 0]  = cumsum_p cost[p, 0]
    dtw[p, w]  = cost[p, w] + min(dtw[p, w-1], dtw[p-1, w-1], dtw[p-1, w])
    out        = sqrt(dtw[:, -32:]) / 32

Strategy (v2; baseline was 108.3us):
  - Data-parallel over batch n: each of 8 cores owns n_loc = 8 rows of x,
    patterns replicated. Per core, two problem tiles of 128 partitions
    (4 n x 32 k); tile t covers n = 4t..4t+3.
  - Cost via one augmented K=10 *fp16* matmul per (q, t): single HW pass
    (fp32 ran LOW/HIGH double-pumped at 2.1us/matmul; fp16 measures 427ns).
    lhs columns ordered so slice q holds patterns p == q (mod 8) with
    partition block b = p // 8. PSUM fp32 -> relu-evict into mm_big fp32
    with free layout (nn, q, w). Tile0's evicts split ACT/DVE to shorten
    the critical path to the first regroup DMA.
  - Regroup (b,k) -> (nn,k) partitions: 32 SBUF->SBUF DMAs (t, nn, b) of
    [32 parts x 4 KB] contiguous on both sides (the p = b*8+q column order
    makes each DMA cover 8 *consecutive* DTW rows, so DMA order matches
    scan consumption order). All triggers on the idle SP sequencer
    (~640ns each measured); the ACT-issued 64-trigger scheme of the
    baseline burned 38us of ACT sequencer time.
  - DTW rows in fp32 (fp16 does NOT speed tensor_tensor_scan - it is
    carry-bound at ~2 cycles/elem either way; measured 397ns vs 397ns).
    D rows are stored at stride W+1 with a BIG guard column before each
    row: the shifted-min tensor_tensor then produces m[0] = D[p-1,0]
    without a separate copy, and the scan's initial is the immediate BIG
    (elem0 = min(m[0], BIG) + c[0] = D[p-1,0] + c[0]), avoiding the
    per-scan initial-AP read (~85ns/scan).
  - Every ISA instruction has ONE sync-wait slot: regroup-DMA completion
    waits ride along on earlier DVE ops whose own deps are same-engine
    (manual add_dep_helper), with tiny tensor_copy absorbers only for the
    first group of each tile; SP nops absorb the relu-completion waits
    ahead of the DMA trigger batches; a tail nop-chain feeds every
    proc's final tick to the sync sequencer so the drain's wait elides.
"""

import numpy as np

import concourse.bass as bass
import concourse.mybir as mybir
import concourse.tile as tile
from concourse.bass_utils import run_bass_kernel_spmd

N, D, W = 64, 8, 128      # x: (N, D, W)
K, P = 32, 32             # patts: (K, D, P)
WO = 32                   # output keeps last WO columns of the DTW table
NCORES = 8
NLOC = N // NCORES        # 8 batch rows per core
NT = 2                    # problem tiles per core: (4 n x 32 k) = 128 partitions
KAUG = D + 2              # augmented contraction dim
BIG = 1e30
WG = W + 1                # row pitch in D (guard column + W data columns)

f32 = mybir.dt.float32
f16 = mybir.dt.float16


def build_program() -> bass.Bass:
    from concourse.tile import add_dep_helper

    nc = bass.Bass()
    inp_d = nc.dram_tensor("inp", (KAUG, K * P + NLOC * W), f16, kind="ExternalInput")
    out_d = nc.dram_tensor("out", (NLOC, K, P, WO), f32, kind="ExternalOutput")

    with tile.TileContext(nc) as tc:
        with (
            tc.tile_pool(name="consts", bufs=1) as consts,
            tc.tile_pool(name="psum", bufs=8, space="PSUM") as psum_pool,
            tc.tile_pool(name="work", bufs=1) as work,
        ):
            inp_s = consts.tile([KAUG, K * P + NLOC * W], f16)
            nc.sync.dma_start(out=inp_s, in_=inp_d[:, :])
            lhs_s = inp_s[:, 0:K * P]
            rhs_s = inp_s[:, K * P:K * P + NLOC * W]
            facta = consts.tile([1, 1], f32)
            factd = [
                consts.tile([1, 1], f32, name=f"factd{i}", tag=f"factd{i}")
                for i in range(4 * 4 * NT)
            ]

            mm_big = [
                work.tile([128, 4 * 8 * W], f32, tag=f"mmb{t}", name=f"mmb{t}")
                for t in range(NT)
            ]
            C2 = [
                work.tile([128, P * W], f32, tag=f"C{t}", name=f"C{t}")
                for t in range(NT)
            ]
            # D table: both tiles in ONE tile so future AP tricks can span
            # them; row p of tile t at cols t*P*WG + p*WG + 1 .. +W, guard
            # (BIG) at t*P*WG + p*WG.
            Dt = work.tile([128, NT * P * WG], f32, tag="D", name="D")
            mt_big = work.tile([128, W], f32, tag="mtb", name="mtb")
            mts = [
                work.tile([128, W], f32, tag=f"mt{t}", name=f"mt{t}")
                for t in range(NT)
            ]

            # Guard memsets on DVE so every TT's guard-read dep coalesces
            # with its same-engine row dep into one sem wait.
            nc.vector.memset(mt_big, BIG)
            for t in range(NT):
                gv = Dt[:, t * P * WG:(t + 1) * P * WG].rearrange(
                    "q (p w) -> q p w", p=P)
                nc.vector.memset(gv[:, :, 0:1], BIG)

            # --- matmuls + evicts: t0 q0..7 then t1 q0..7.  t0's evicts
            # split ACT (q0..3) / DVE (q4..7) to finish ~2.5us earlier.
            relu_acts = {t: [] for t in range(NT)}   # ACT evicts per tile
            relu_dves = {t: [] for t in range(NT)}
            last_mm = None
            act_fence = None

            def emit_tile_mms(t):
                nonlocal last_mm
                mmv = mm_big[t].rearrange("q (nn g w) -> q nn g w", nn=4, g=8)
                for q in range(8):
                    ps = psum_pool.tile([128, 512], f32)
                    last_mm = nc.tensor.matmul(
                        ps,
                        lhs_s[:, q * 128:(q + 1) * 128],
                        rhs_s[:, t * 512:(t + 1) * 512],
                        start=True,
                        stop=True,
                    )
                    if t == 0 and q >= 4:
                        ev = nc.vector.tensor_scalar_max(mmv[:, :, q, :], ps, 0.0)
                        relu_dves[t].append(ev)
                    else:
                        ev = nc.scalar.activation(
                            mmv[:, :, q, :], ps,
                            mybir.ActivationFunctionType.Relu,
                        )
                        relu_acts[t].append(ev)

            emit_tile_mms(0)
            emit_tile_mms(1)

            # --- regroup DMAs (t, nn, b): mm_big[t][b-block parts,
            # nn-block free] -> C2[t][nn-block parts, rows b*8..b*8+7].
            # Tile0's 16 trigger on the idle SP sequencer; the first one
            # sync-deps the ACT fence (drops its direct DVE-evict edges,
            # keeps one coalesced ACT wait), the second carries the single
            # coalesced DVE wait, later ones elide via SP's wait clock and
            # carry only HWDGE slot-reuse waits.  Tile1's 16 trigger on
            # gpsimd (SWDGE): a separate queue space, and the Pool engine
            # is otherwise idle; the first carries the coalesced ACT wait
            # for t1's evicts.
            def emit_group(t, b):
                eng = nc.sync if t == 0 else nc.gpsimd
                out = []
                for nn in range(4):
                    dma = eng.dma_start(
                        out=C2[t][nn * 32:(nn + 1) * 32,
                                  b * 8 * W:(b + 1) * 8 * W],
                        in_=mm_big[t][b * 32:(b + 1) * 32,
                                      nn * 8 * W:(nn + 1) * 8 * W],
                    )
                    out.append(dma)
                return out

            # SP pre-DMA reading a q7 (DVE-evicted) corner: carries the
            # single coalesced DVE wait and registers it in SP's wait
            # clock, so the real regroup DMAs' DVE-evict deps elide and
            # each carries at most the coalesced ACT wait / a slot wait.
            scrap = consts.tile([1, 64], f32, name="scrap", tag="scrap")
            nc.sync.dma_start(out=scrap, in_=mm_big[0][0:1, 7 * W:7 * W + 64])
            groups = {}
            for b in range(4):
                groups[(0, b)] = emit_group(0, b)
            for b in range(4):
                groups[(1, b)] = emit_group(1, b)
            all_dmas = [d for g in groups.values() for d in g]

            # --- DTW scans.  Schedule: t0 rows 0..7 solo, then lag-8
            # interleave, then t1 rows 24..31 solo.
            sched = [(0, p) for p in range(8)]
            for i in range(24):
                sched.append((1, i))
                sched.append((0, i + 8))
            sched += [(1, p) for p in range(24, 32)]

            nfact = 0
            last_scan = {}
            group_anchor = None

            for (t, p) in sched:
                base = t * P * WG + p * WG
                cr = C2[t][:, p * W:(p + 1) * W]
                if p % 8 == 0:
                    # 4 absorber copies, one per regroup DMA of this group
                    # (corner read -> auto-dep, one DMA-sem wait each);
                    # nosync edges chain them and anchor the first scan.
                    b = p // 8
                    prev_ab = None
                    for i in range(4):
                        ab = nc.vector.tensor_copy(
                            factd[nfact],
                            C2[t][i * 32:i * 32 + 1, b * 8 * W:b * 8 * W + 1],
                        )
                        nfact += 1
                        if prev_ab is not None:
                            add_dep_helper(ab.ins, prev_ab.ins, sync=False,
                                           reason="absorber chain order")
                        prev_ab = ab
                    group_anchor = prev_ab
                if p == 0:
                    scan = nc.vector.tensor_tensor_scan(
                        Dt[:, base + 1:base + 1 + W], mt_big, cr, 0.0,
                        mybir.AluOpType.min, mybir.AluOpType.add,
                    )
                else:
                    pbase = t * P * WG + (p - 1) * WG
                    mt = mts[t]
                    nc.vector.tensor_tensor(
                        mt, Dt[:, pbase:pbase + W],
                        Dt[:, pbase + 1:pbase + 1 + W],
                        mybir.AluOpType.min,
                    )
                    scan = nc.vector.tensor_tensor_scan(
                        Dt[:, base + 1:base + 1 + W], mt, cr, BIG,
                        mybir.AluOpType.min, mybir.AluOpType.add,
                    )
                if p % 8 == 0:
                    add_dep_helper(scan.ins, group_anchor.ins, sync=False,
                                   reason="scan after DVE absorbers")
                last_scan[t] = scan

            # --- sqrt + out DMA per tile (ACT), overlapped with the other
            # tile's remaining scans.
            odmas, ofences = [], []
            for t in range(NT):
                ot = work.tile([128, P, WO], f32, name=f"ot{t}", tag=f"ot{t}")
                dv = Dt[:, t * P * WG:(t + 1) * P * WG].rearrange(
                    "q (p w) -> q p w", p=P)
                nc.scalar.activation(
                    ot[:, :, :], dv[:, :, WG - WO:WG],
                    mybir.ActivationFunctionType.Sqrt,
                    scale=1.0 / (P * P),
                )
                ofence = nc.scalar.activation(
                    facta, ot[0:1, P - 1, 0:1],
                    mybir.ActivationFunctionType.Copy,
                )
                odma = nc.scalar.dma_start(
                    out=out_d[t * 4:(t + 1) * 4, :, :, :], in_=ot
                )
                add_dep_helper(odma.ins, ofence.ins, sync=False,
                               reason="out DMA after ACT wait-absorber")
                odmas.append(odma)
                ofences.append(ofence)

            # --- tail: feed every proc's final tick into the sync
            # sequencer so the kernel-tail drain's wait list elides.
            tail_deps = (all_dmas[8:16] + all_dmas[-8:] + odmas
                         + [ofences[-1], last_mm]
                         + [last_scan[t] for t in range(NT)])
            prev_nop = None
            for td in tail_deps:
                nop = nc.sync.nop()
                add_dep_helper(nop.ins, td.ins, sync=True,
                               reason="drain pre-absorber")
                if prev_nop is not None:
                    add_dep_helper(nop.ins, prev_nop.ins, sync=False,
                                   reason="keep nop chain ordered")
                prev_nop = nop
    return nc


def make_in_maps(x: np.ndarray, patts: np.ndarray) -> list[dict[str, np.ndarray]]:
    x = np.ascontiguousarray(x, dtype=np.float32)
    patts = np.ascontiguousarray(patts, dtype=np.float32)
    # lhs columns: col = q*128 + b*32 + k  <->  pattern p = b*8 + q
    pf = patts.transpose(1, 2, 0)                     # (d, P, K)
    p2f = (patts * patts).sum(axis=1).T               # (P, K)
    lhs = np.empty((KAUG, P * K), np.float32)
    cols = np.arange(P * K)
    q, b, k = cols // 128, (cols % 128) // 32, cols % 32
    p = b * 8 + q
    lhs[:D, :] = -2.0 * pf[:, p, k]
    lhs[D, :] = p2f[p, k]
    lhs[D + 1, :] = 1.0
    lhs16 = lhs.astype(np.float16)

    in_maps = []
    for c in range(NCORES):
        xs = x[c * NLOC:(c + 1) * NLOC]                          # (8, 8, 128)
        xf = xs.transpose(1, 0, 2).reshape(D, NLOC * W)          # [d, (n w)]
        x2 = (xs * xs).sum(axis=1).reshape(1, NLOC * W)
        rhs = np.concatenate(
            [xf, np.ones((1, NLOC * W), np.float32), x2], axis=0)
        in_maps.append({"inp": np.concatenate(
            [lhs16, rhs.astype(np.float16)], axis=1)})
    return in_maps


_program_cache: bass.Bass | None = None


def kernel(x: np.ndarray, patts: np.ndarray) -> np.ndarray:
    global _program_cache
    if _program_cache is None:
        _program_cache = build_program()
    nc = _program_cache
    in_maps = make_in_maps(x, patts)
    res = run_bass_kernel_spmd(nc, in_maps, list(range(NCORES)))
    return np.concatenate([r["out"] for r in res.results], axis=0)


if __name__ == "__main__":
    rng = np.random.default_rng(0)
    x = rng.standard_normal((N, D, W), dtype=np.float32)
    patts = rng.standard_normal((K, D, P), dtype=np.float32)
    out = kernel(x, patts)
    print(out.shape, out.dtype)



# revision 16
# speedup vs baseline: 1.1120x; 1.1120x over previous
"""DTW frames layer on 8 Trainium2 NeuronCores.

Reference computation (per (n, k) problem):
    cost[p, w] = max(0, ||x[n, :, w] - patts[k, :, p]||^2)          (P=32, W=128)
    dtw[0, w]  = cumsum_w cost[0, w]
    dtw[p, 0]  = cumsum_p cost[p, 0]
    dtw[p, w]  = cost[p, w] + min(dtw[p, w-1], dtw[p-1, w-1], dtw[p-1, w])
    out        = sqrt(dtw[:, -32:]) / 32

Strategy (v4; v2 was 71.9us, v1 108.3us):
  - Data-parallel over batch n: each of 8 cores owns n_loc = 8 rows of x,
    patterns replicated. Per core, two problem tiles of 128 partitions
    (4 n x 32 k); tile t covers n = 4t..4t+3.
  - Cost via one augmented K=10 fp16 matmul per (q, t) (427ns each).
    lhs columns ordered so slice q holds patterns p == q (mod 8) with
    partition block b = p // 8. PSUM fp32 -> relu-evict into mm_big FP16
    with free layout (nn, q, w). t0 evicts split ACT (q0-3) / DVE (q4-7);
    t1 evicts all on ACT.
  - Regroup (b,k) -> (nn,k) partitions in FP16 (half the DMA bytes of
    v2): ONE merged dma_start per (t, b) group with a [kk, nn, x] AP
    (the nn partition-block loop folds into the DMA), so each group
    costs one ~565ns trigger and posts ONE queue sem.  t0's 4 groups
    trigger on SP (scrap pre-DMA absorbs the DVE-evict wait); t1's 4 on
    ACT, where they are pure program order after ACT's t1 evicts.
  - DTW rows fp32, stride W+1 with a BIG guard column before each row
    (scan initial = BIG; elem0 = D[p-1,0] + c[0] without an initial-AP
    read).  The scan's in1 (cost) is fp16, in0/out fp32; the scan state
    is fp32 in hardware regardless.  Pool/GPSIMD cannot run TensorTensor
    min (ucode unimplemented), so the shifted-min stays on DVE,
    interleaved across the two tile chains (~517ns/row).
  - Schedule: t0 rows 0..LAG-1 solo, lag-LAG interleave, t1 tail solo.
    LAG=6 matches when t1's first regrouped block lands (~3.5us after
    t0's).
  - Output: per-group (8 rows) sqrt on ACT + out DMA in a group-major
    DRAM layout ([g, part, q, w], host reorders) so the last group's
    tail is ~1/4 of a full-tile tail and earlier groups stream out
    during the scan phase.
  - Wait hygiene (one sem-wait slot per instruction; elision is
    per-engine): each group gets one DVE absorber copy carrying the
    merged group DMA's queue sem; scans then ride DVE's wait clock.
    Tail nop-chain feeds final ticks to SP so the drain elides.
"""

import numpy as np

import concourse.bass as bass
import concourse.mybir as mybir
import concourse.tile as tile
from concourse.bass_utils import run_bass_kernel_spmd

N, D, W = 64, 8, 128      # x: (N, D, W)
K, P = 32, 32             # patts: (K, D, P)
WO = 32                   # output keeps last WO columns of the DTW table
NCORES = 8
NLOC = N // NCORES        # 8 batch rows per core
NT = 2                    # problem tiles per core: (4 n x 32 k) = 128 partitions
KAUG = D + 2              # augmented contraction dim
BIG = 1e30
WG = W + 1                # row pitch in D (guard column + W data columns)
LAG = 6                   # t1 joins the interleave after LAG t0 rows
NG = 4 * NT               # output groups (8 rows each)

f32 = mybir.dt.float32
f16 = mybir.dt.float16


def build_program() -> bass.Bass:
    from concourse.tile import add_dep_helper

    nc = bass.Bass()
    inp_d = nc.dram_tensor("inp", (KAUG, K * P + NLOC * W), f16, kind="ExternalInput")
    out_d = nc.dram_tensor("out", (NG, 128, 8, WO), f32, kind="ExternalOutput")

    with tile.TileContext(nc) as tc:
        with (
            tc.tile_pool(name="consts", bufs=1) as consts,
            tc.tile_pool(name="psum", bufs=8, space="PSUM") as psum_pool,
            tc.tile_pool(name="work", bufs=1) as work,
        ):
            inp_s = consts.tile([KAUG, K * P + NLOC * W], f16)
            inp_dma = nc.sync.dma_start(out=inp_s, in_=inp_d[:, :])
            lhs_s = inp_s[:, 0:K * P]
            rhs_s = inp_s[:, K * P:K * P + NLOC * W]

            mm_big = [
                work.tile([128, 4 * 8 * W], f16, tag=f"mmb{t}", name=f"mmb{t}")
                for t in range(NT)
            ]
            C2 = [
                work.tile([128, P * W], f16, tag=f"C{t}", name=f"C{t}")
                for t in range(NT)
            ]
            # D table: row p of tile t at cols t*P*WG + p*WG + 1 .. +W, guard
            # (BIG) at t*P*WG + p*WG.
            Dt = work.tile([128, NT * P * WG], f32, tag="D", name="D")
            mt_big = work.tile([128, W], f32, tag="mtb", name="mtb")
            # One mt slice per (t, p) row: no reuse, no WAR/WAW chains.
            mtbuf = work.tile([128, NT * P * W], f32, tag="mtf", name="mtf")

            # Guard memsets on DVE so every DVE consumer's guard-read dep is
            # same-engine program order (free).
            nc.vector.memset(mt_big, BIG)
            for t in range(NT):
                gv = Dt[:, t * P * WG:(t + 1) * P * WG].rearrange(
                    "q (p w) -> q p w", p=P)
                nc.vector.memset(gv[:, :, 0:1], BIG)

            # --- matmuls + evicts: t0 q0..7 then t1 q0..7.  t0's evicts
            # split ACT (q0..3) / DVE (q4..7); t1's all on ACT so the
            # ACT-triggered t1 regroup DMAs need no sem waits at all.
            last_mm = None

            def emit_tile_mms(t):
                nonlocal last_mm
                mmv = mm_big[t].rearrange("q (nn g w) -> q nn g w", nn=4, g=8)
                for q in range(8):
                    ps = psum_pool.tile([128, 512], f32)
                    last_mm = nc.tensor.matmul(
                        ps,
                        lhs_s[:, q * 128:(q + 1) * 128],
                        rhs_s[:, t * 512:(t + 1) * 512],
                        start=True,
                        stop=True,
                    )
                    if t == 0 and q >= 4:
                        nc.vector.tensor_scalar_max(mmv[:, :, q, :], ps, 0.0)
                    else:
                        nc.scalar.activation(
                            mmv[:, :, q, :], ps,
                            mybir.ActivationFunctionType.Relu,
                        )

            emit_tile_mms(0)
            emit_tile_mms(1)

            # --- regroup: ONE merged DMA per (t, b) group.
            #   C2[t][part 32*nn+kk, (b*8+q)*W + w] =
            #       mm_big[t][part 32*b+kk, (nn*8 + q)*W + w]
            def emit_group(t, b):
                eng = nc.sync if t == 0 else nc.scalar
                out = []
                for nn in range(4):
                    dma = eng.dma_start(
                        out=C2[t][nn * 32:(nn + 1) * 32,
                                  b * 8 * W:(b + 1) * 8 * W],
                        in_=mm_big[t][b * 32:(b + 1) * 32,
                                      nn * 8 * W:(nn + 1) * 8 * W],
                    )
                    out.append(dma)
                return out

            # SP pre-DMA reading a q7 (DVE-evicted) corner: carries the
            # single coalesced DVE wait and registers it in SP's wait
            # clock, so the t0 group DMAs carry at most the coalesced
            # ACT wait.
            scrap = consts.tile([1, 64], f16, name="scrap", tag="scrap")
            scrap_dma = nc.sync.dma_start(
                out=scrap, in_=mm_big[0][0:1, 7 * W:7 * W + 64])
            groups = {}
            for b in range(4):
                groups[(0, b)] = emit_group(0, b)
            for b in range(4):
                groups[(1, b)] = emit_group(1, b)
            ring_preds = ([inp_dma, scrap_dma]
                          + [d for t in range(NT) for b in range(4)
                             for d in groups[(t, b)]])
            ring_fenced = [False]

            # --- DTW scans.  Schedule: t0 rows 0..LAG-1 solo, lag-LAG
            # interleave, t1 tail solo.  All scans and mins on DVE.
            sched = [(0, p) for p in range(LAG)]
            for i in range(P - LAG):
                sched.append((1, i))
                sched.append((0, i + LAG))
            sched += [(1, p) for p in range(P - LAG, P)]

            # Scratch targets for the per-group DVE absorbers and ACT
            # ofence copies.
            gabs = [
                consts.tile([1, 1], f16, name=f"gab{i}", tag=f"gab{i}")
                for i in range(NG * 4)
            ]
            gabs2 = [
                consts.tile([1, 1], f32, name=f"gob{i}", tag=f"gob{i}")
                for i in range(40)
            ]

            last_scan = {}

            for (t, p) in sched:
                base = t * P * WG + p * WG
                cr = C2[t][:, p * W:(p + 1) * W]
                if p % 8 == 0:
                    # Four DVE absorbers per group (one per regroup DMA):
                    # each carries one queue sem in DVE's wait clock, so
                    # the group's scans keep their single wait slot for
                    # the DVE self-wait chain.
                    b = p // 8
                    prev_ab = None
                    for i4 in range(4):
                        ab = nc.vector.tensor_copy(
                            gabs[(t * 4 + b) * 4 + i4],
                            C2[t][i4 * 32:i4 * 32 + 1,
                                  b * 8 * W:b * 8 * W + 1],
                        )
                        if prev_ab is not None:
                            add_dep_helper(ab.ins, prev_ab.ins, sync=False,
                                           reason="absorber chain order")
                        prev_ab = ab
                if p == 0:
                    scan = nc.vector.tensor_tensor_scan(
                        Dt[:, base + 1:base + 1 + W], mt_big, cr, 0.0,
                        mybir.AluOpType.min, mybir.AluOpType.add,
                    )
                else:
                    pbase = t * P * WG + (p - 1) * WG
                    mi = (t * P + p) * W
                    mt = mtbuf[:, mi:mi + W]
                    nc.vector.tensor_tensor(
                        mt, Dt[:, pbase:pbase + W],
                        Dt[:, pbase + 1:pbase + 1 + W],
                        mybir.AluOpType.min,
                    )
                    scan = nc.vector.tensor_tensor_scan(
                        Dt[:, base + 1:base + 1 + W], mt, cr, BIG,
                        mybir.AluOpType.min, mybir.AluOpType.add,
                    )
                last_scan[t] = scan

                # Group (t, b) finished?  Emit its sqrt + out DMA on ACT.
                if p % 8 == 7:
                    b = p // 8
                    g = t * 4 + b
                    ot = work.tile([128, 8, WO], f32, name=f"ot{g}",
                                   tag=f"ot{g}")
                    dv = Dt[:, t * P * WG:(t + 1) * P * WG].rearrange(
                        "z (pp w) -> z pp w", pp=P)
                    nc.scalar.activation(
                        ot[:, :, :], dv[:, b * 8:(b + 1) * 8, WG - WO:WG],
                        mybir.ActivationFunctionType.Sqrt,
                        scale=1.0 / (P * P),
                    )
                    # Ring fences (once, before the first out DMA): ACT
                    # copies that wait every earlier DMA's ring sem, so all
                    # odmas' ring-reuse waits elide via ACT's wait clock and
                    # each odma's single wait slot holds the REAL data dep
                    # (the sqrt's Activation sem).  Robust to scheduler
                    # reordering of the ring round-robin.
                    if not ring_fenced[0]:
                        ring_fenced[0] = True
                        prev_f = None
                        for j, pred in enumerate(ring_preds):
                            rf = nc.scalar.activation(
                                gabs2[j], gabs2[j],
                                mybir.ActivationFunctionType.Copy,
                            )
                            add_dep_helper(rf.ins, pred.ins, sync=True,
                                           reason="absorb ring-reuse wait")
                            if prev_f is not None:
                                add_dep_helper(rf.ins, prev_f.ins, sync=False,
                                               reason="fence chain order")
                            prev_f = rf
                        ring_fenced.append(prev_f)
                    odma = nc.scalar.dma_start(out=out_d[g, :, :, :], in_=ot)
                    add_dep_helper(odma.ins, ring_fenced[1].ins, sync=False,
                                   reason="odma after ring fences")
                    groups[("out", g)] = odma

            # --- tail: feed every proc's final tick into the sync
            # sequencer so the kernel-tail drain's wait list elides.
            gflat = []
            for v in groups.values():
                gflat.extend(v if isinstance(v, list) else [v])
            tail_deps = (gflat
                         + [last_mm]
                         + [last_scan[t] for t in range(NT)])
            prev_nop = None
            for td in tail_deps:
                nop = nc.sync.nop()
                add_dep_helper(nop.ins, td.ins, sync=True,
                               reason="drain pre-absorber")
                if prev_nop is not None:
                    add_dep_helper(nop.ins, prev_nop.ins, sync=False,
                                   reason="keep nop chain ordered")
                prev_nop = nop
    return nc


def make_in_maps(x: np.ndarray, patts: np.ndarray) -> list[dict[str, np.ndarray]]:
    x = np.ascontiguousarray(x, dtype=np.float32)
    patts = np.ascontiguousarray(patts, dtype=np.float32)
    # lhs columns: col = q*128 + b*32 + k  <->  pattern p = b*8 + q
    pf = patts.transpose(1, 2, 0)                     # (d, P, K)
    p2f = (patts * patts).sum(axis=1).T               # (P, K)
    lhs = np.empty((KAUG, P * K), np.float32)
    cols = np.arange(P * K)
    q, b, k = cols // 128, (cols % 128) // 32, cols % 32
    p = b * 8 + q
    lhs[:D, :] = -2.0 * pf[:, p, k]
    lhs[D, :] = p2f[p, k]
    lhs[D + 1, :] = 1.0
    lhs16 = lhs.astype(np.float16)

    in_maps = []
    for c in range(NCORES):
        xs = x[c * NLOC:(c + 1) * NLOC]                          # (8, 8, 128)
        xf = xs.transpose(1, 0, 2).reshape(D, NLOC * W)          # [d, (n w)]
        x2 = (xs * xs).sum(axis=1).reshape(1, NLOC * W)
        rhs = np.concatenate(
            [xf, np.ones((1, NLOC * W), np.float32), x2], axis=0)
        in_maps.append({"inp": np.concatenate(
            [lhs16, rhs.astype(np.float16)], axis=1)})
    return in_maps


_program_cache: bass.Bass | None = None


def kernel(x: np.ndarray, patts: np.ndarray) -> np.ndarray:
    global _program_cache
    if _program_cache is None:
        _program_cache = build_program()
    nc = _program_cache
    in_maps = make_in_maps(x, patts)
    res = run_bass_kernel_spmd(nc, in_maps, list(range(NCORES)))
    outs = []
    for r in res.results:
        v = r["out"].reshape(NT, 4, 4, K, 8, WO)      # [t, b, nn, k, q, w]
        outs.append(v.transpose(0, 2, 3, 1, 4, 5).reshape(NLOC, K, P, WO))
    return np.concatenate(outs, axis=0)


if __name__ == "__main__":
    rng = np.random.default_rng(0)
    x = rng.standard_normal((N, D, W), dtype=np.float32)
    patts = rng.standard_normal((K, D, P), dtype=np.float32)
    out = kernel(x, patts)
    print(out.shape, out.dtype)
